# revision 15
# baseline (speedup 1.0000x reference)
"""Depth-to-space (pixel shuffle / DUC) kernel for Trainium2.

Full op: x[16, 1216, 32, 32] f32 -> out[16, 19, 304, 304] f32 where
  out[b, c, i*8+r1, j*8+r2] = x[b, c*64 + r1*8 + r2, i, j]
and out is zero-padded from 256x256 up to 304x304 (bottom/right).

Sharding: pure data-parallel over batch, 2 examples per core on 8 cores.

The op is pure data movement; at this size the dominant cost is the
per-DMA-instruction descriptor-generation overhead (~0.5 us each,
HW-measured), so the layout is chosen to minimize DMA instruction count
while keeping every DRAM-side descriptor >= 128B contiguous:

  Tile layout (2 images of one batch share a [128, 1024] f32 tile):
    partition p = c2*64 + i*2 + s   (c2 = image within pair, s = r1>>2)
    raw free    = t*32 + j          (t = local channel = (r1&3)*8 + r2)
  - Load: for fixed (image, s) the needed 32 channels are consecutive in
    DRAM and their free offsets are uniformly strided -> one 3-dim DMA
    per (image, s): 2 loads/image, 128B descriptors.
  - Shuffle: output row y = i*8+r1 = 4*(i*2+s) + (r1&3), so each
    partition holds 4 output rows; one DVE strided copy per pair-tile
    transposes [r2][j] -> [j][r2] within each 1KB row segment.
  - Store: one 3-dim DMA per image (dims p, u, w), 1KB descriptors.

Per core: 76 loads + 38 stores = 114 DMA instructions, 20 DVE copies.
Zero padding relies on ExternalOutput buffers being pre-zeroed by the
runner (both the native path and the PJRT/axon path guarantee this).
"""

import sys

if "/opt/trn_rl_repo" not in sys.path:
    sys.path.insert(0, "/opt/trn_rl_repo")

import numpy as np

B, CLASSES, R, H, W, OUT = 16, 19, 8, 32, 32, 304
HR = H * R  # 256
N_CORES = 8
BPC = B // N_CORES  # batches per core = 2

_NC_CACHE = {}


def build_nc(
    bpc=BPC,
    classes=CLASSES,
    zero_init=False,
    num_devices=N_CORES,
    repeats=1,
    loop_repeats=1,
    load_engines=("sync", "scalar"),
    store_engines=("scalar", "sync"),
    copy_engines=("vector",),
    bufs=4,
):
    import concourse.bacc as bacc
    import concourse.mybir as mybir
    from concourse.tile import TileContext

    f32 = mybir.dt.float32
    # Bacc (not plain Bass): its compile() legalizes multi-sem sync waits
    # that walrus otherwise rejects ("Too many sync wait commands").
    nc = bacc.Bacc(
        "TRN2", target_bir_lowering=False, debug=False, num_devices=num_devices
    )
    x = nc.declare_dram_parameter("x", [bpc, classes * R * R, H, W], f32, isOutput=False)
    out = nc.declare_dram_parameter("out", [bpc, classes, OUT, OUT], f32, isOutput=True)

    def eng(name):
        return getattr(nc, name)

    n_load = 0
    n_store = 0
    n_copy = 0
    with TileContext(nc) as tc:
        with (
            tc.tile_pool(name="raw", bufs=bufs) as raw_pool,
            tc.tile_pool(name="row", bufs=bufs) as row_pool,
        ):
          def _do_pair(b, c0, n_img):
            """Emit loads/shuffle/stores for images (b, c0..c0+n_img-1)."""
            nonlocal n_load, n_store, n_copy
            raw = raw_pool.tile([128, 4 * HR], f32)
            row = row_pool.tile([128, 4 * HR], f32)
            if zero_init:
                nc.gpsimd.memset(raw[:], 0.0)
            # partition p = c2*64 + i*2 + s ; free = t*32 + j
            raw_v = raw[:].rearrange("(c2 i s) f -> c2 s i f", c2=2, s=2)
            for c2 in range(n_img):
                c = c0 + c2
                for s in range(2):
                    # 32 consecutive channels c*64 + s*32 + t, t=0..31
                    src = x[b, c * 64 + s * 32 : c * 64 + (s + 1) * 32]
                    src = src.rearrange("t i j -> i t j")  # (i, t, j)
                    dst = raw_v[c2, s].rearrange("i (t j) -> i t j", t=32)
                    eng(load_engines[n_load % len(load_engines)]).dma_start(
                        out=dst, in_=src
                    )
                    n_load += 1
            npart = 64 * n_img
            # shuffle t*32+j = (u*8+r2)*32+j  ->  u*256 + j*8 + r2
            s2 = raw[0:npart].rearrange("p (u r2 j) -> p u j r2", u=4, r2=R)
            d2 = row[0:npart].rearrange("p (u j r2) -> p u j r2", u=4, r2=R)
            getattr(nc, copy_engines[n_copy % len(copy_engines)]).tensor_copy(d2, s2)
            n_copy += 1
            # store: rows y = 4*(i*2+s) + u; dims (p, u, w), 1KB runs
            for c2 in range(n_img):
                c = c0 + c2
                eng(store_engines[n_store % len(store_engines)]).dma_start(
                    out=out[b, c, 0:HR, 0:HR].rearrange("(p u) w -> p u w", u=4),
                    in_=row[c2 * 64 : (c2 + 1) * 64].rearrange("p (u w) -> p u w", u=4),
                )
                n_store += 1

          def _body():
            for b in range(bpc):
                for c0 in range(0, classes - 1, 2):
                    _do_pair(b, c0, 2)
                if classes % 2:
                    _do_pair(b, classes - 1, 1)

          if loop_repeats > 1:
              # measurement-only: on-device loop to amortize dispatch noise
              with tc.For_i(0, loop_repeats, 1):
                  _body()
          else:
              for _rep in range(repeats):
                  _body()
    nc.compile()
    return nc


def _get_nc():
    key = "main"
    if key not in _NC_CACHE:
        _NC_CACHE[key] = build_nc()
    return _NC_CACHE[key]


def kernel(x: np.ndarray) -> np.ndarray:
    from concourse.bass_utils import run_bass_kernel_spmd

    x = np.ascontiguousarray(x, dtype=np.float32)
    assert x.shape == (B, CLASSES * R * R, H, W), x.shape
    nc = _get_nc()
    in_maps = [{"x": x[k * BPC : (k + 1) * BPC]} for k in range(N_CORES)]
    res = run_bass_kernel_spmd(nc, in_maps, list(range(N_CORES)))
    return np.concatenate([res.results[k]["out"] for k in range(N_CORES)], axis=0)
